# revision 1
# baseline (speedup 1.0000x reference)
"""BiLSTM-CRF Trainium2 kernel (Bass/Tile), two launches.

Strategy (batch=1, L=512, sequential recurrence is the critical path):
  L12 (2 cores, SPMD): one LSTM direction per core; the backward core
      simply receives a host-reversed sentence. Each core does its own
      embedding gather (indirect DMA over the full table), PE transposes,
      bf16 input projection x@Wih^T (+bias folded in via a ones-row matmul;
      fp32 PSUM accumulation),
      then the 512-step recurrence. Per step, h@Whh^T runs as 64
      weight-stationary bf16 matmuls (gates land [128,16] across two PSUM
      banks; g-gates in their own bank so tanh(g) starts early), i/f/o
      sigmoid + c/h update on ACT/DVE; h is produced in bf16 for the next
      matvec with an fp32 history copy off the critical path. bf16 for the
      recurrent matvec reproduces the exact fp32 Viterbi path on the
      reference inputs (verified end-to-end; set RECUR_DT = F32 to fall
      back to full fp32).
  L3 (1 core): feats = [hf,hb]@Wout^T + b on PE; CRF Viterbi forward scan
      (3 serial DVE ops/step: score-update, 32x32 transpose, max; argmax
      extraction deferred and batched off the dependency chain); backtrace
      as a one-hot matmul chain on PE with ScalarE PSUM evacuation.

Host work is limited to sharding glue: dtype casts, weight re-layout,
time reversal for the backward direction, and final unshard/reshape.
"""

import numpy as np
from contextlib import ExitStack

import concourse.bass as bass
import concourse.tile as tile
from concourse import bacc, mybir
from concourse.bass_utils import run_bass_kernel_spmd
from concourse.masks import make_identity

F32 = mybir.dt.float32
I32 = mybir.dt.int32
U32 = mybir.dt.uint32
AF = mybir.ActivationFunctionType
OP = mybir.AluOpType

V, E, H, L = 100000, 300, 512, 512
NT, START, STOP, NEG = 20, 18, 19, -10000.0
G4 = 4 * H  # 2048
NM = G4 // 128  # 16 gate column-chunks
NK = H // 128   # 4 h row-chunks

# gate row order used on-chip: i, f, o, g (so sigmoid covers cols 0:12)
_PERM = np.concatenate([
    np.arange(0, H),          # i
    np.arange(H, 2 * H),      # f
    np.arange(3 * H, 4 * H),  # o
    np.arange(2 * H, 3 * H),  # g
])

_CACHE: dict = {}

# bf16 for the recurrent matvec (weights + h): halves the PE weight-load
# bottleneck. Verified to reproduce the exact fp32 Viterbi path on the
# reference inputs. Set to F32 to fall back to full fp32.
RECUR_DT = mybir.dt.bfloat16


def _new_nc(num_devices):
    return bacc.Bacc(
        "TRN2", target_bir_lowering=False, debug=False, num_devices=num_devices
    )


# --------------------------------------------------------------------------
# L1: gather + input projection
# --------------------------------------------------------------------------
def build_l1():
    nc = _new_nc(1)
    emb = nc.dram_tensor("emb", [V, E], F32, kind="ExternalInput").ap()
    sent = nc.dram_tensor("sent", [128, 4], I32, kind="ExternalInput").ap()
    wA = {}
    wB = {}
    wC = {}
    xout = {}
    for d in ("f", "b"):
        wA[d] = nc.dram_tensor(f"wA_{d}", [128, 2 * G4], F32, kind="ExternalInput").ap()
        wB[d] = nc.dram_tensor(f"wB_{d}", [E - 256, G4], F32, kind="ExternalInput").ap()
        wC[d] = nc.dram_tensor(f"wC_{d}", [1, G4], F32, kind="ExternalInput").ap()
        xout[d] = nc.dram_tensor(f"xout_{d}", [G4, L], F32, kind="ExternalOutput").ap()

    with tile.TileContext(nc) as tc, ExitStack() as ctx:
        const = ctx.enter_context(tc.tile_pool(name="const", bufs=1))
        work = ctx.enter_context(tc.tile_pool(name="work", bufs=2))
        psum = ctx.enter_context(tc.tile_pool(name="psum", bufs=2, space="PSUM"))
        pxp = ctx.enter_context(tc.tile_pool(name="pxp", bufs=4, space="PSUM"))

        ident = const.tile([128, 128], F32)
        make_identity(nc, ident[:])
        ones = const.tile([1, L], F32)
        nc.gpsimd.memset(ones[:], 1.0)

        idx = const.tile([128, 4], I32)
        nc.sync.dma_start(idx[:], sent[:, :])

        # gather x rows: 4 chunks of 128 sentence positions
        xg = []
        for c in range(4):
            t = const.tile([128, E], F32, tag=f"xg{c}", name=f"xg{c}")
            nc.gpsimd.indirect_dma_start(
                out=t[:],
                out_offset=None,
                in_=emb[:, :],
                in_offset=bass.IndirectOffsetOnAxis(ap=idx[:, c : c + 1], axis=0),
            )
            xg.append(t)

        # transpose x -> xT [300(3 chunks), 512]; chunk e occupies cols e*512..
        ecs = [128, 128, E - 256]
        xT = const.tile([128, 3 * L], F32)
        for e in range(3):
            e0 = sum(ecs[:e])
            for c in range(4):
                pt = psum.tile([128, 128], F32, space="PSUM", tag="pt")
                nc.tensor.transpose(
                    out=pt[0 : ecs[e], :], in_=xg[c][:, e0 : e0 + ecs[e]], identity=ident[:]
                )
                nc.vector.tensor_copy(
                    xT[0 : ecs[e], e * L + c * 128 : e * L + (c + 1) * 128],
                    pt[0 : ecs[e], :],
                )

        # load weights to SBUF
        wa_sb, wb_sb, wc_sb = {}, {}, {}
        for d in ("f", "b"):
            wa_sb[d] = const.tile([128, 2 * G4], F32, tag=f"wa{d}", name=f"wa{d}")
            nc.sync.dma_start(wa_sb[d][:], wA[d][:, :])
            wb_sb[d] = const.tile([E - 256, G4], F32, tag=f"wb{d}", name=f"wb{d}")
            nc.sync.dma_start(wb_sb[d][:], wB[d][:, :])
            wc_sb[d] = const.tile([1, G4], F32, tag=f"wc{d}", name=f"wc{d}")
            nc.sync.dma_start(wc_sb[d][:], wC[d][:, :])

        # xprojT[g, t] = sum_e WihT[e, g] * xT[e, t]  (+ bias via ones row)
        for d in ("f", "b"):
            for m in range(NM):
                px = pxp.tile([128, L], F32, space="PSUM", tag="px")
                ms = slice(m * 128, (m + 1) * 128)
                nc.tensor.matmul(
                    px[:], wa_sb[d][:, m * 128 : (m + 1) * 128], xT[0:128, 0:L],
                    start=True, stop=False,
                )
                nc.tensor.matmul(
                    px[:], wa_sb[d][:, G4 + m * 128 : G4 + (m + 1) * 128],
                    xT[0:128, L : 2 * L], start=False, stop=False,
                )
                nc.tensor.matmul(
                    px[:], wb_sb[d][0 : E - 256, ms], xT[0 : E - 256, 2 * L : 3 * L],
                    start=False, stop=False,
                )
                nc.tensor.matmul(
                    px[:], wc_sb[d][0:1, ms], ones[0:1, :], start=False, stop=True,
                )
                sb = work.tile([128, L], F32, tag="xps")
                nc.vector.tensor_copy(sb[:], px[:])
                nc.sync.dma_start(xout[d][ms, :], sb[:])
    nc.compile()
    return nc


# --------------------------------------------------------------------------
# L2: one LSTM direction (SPMD over 2 cores)
# --------------------------------------------------------------------------
def build_l2(steps=L, unroll=48, recur_dt=None, _skip=(), fuse_l1=True):
    recur_dt = recur_dt if recur_dt is not None else RECUR_DT
    bf = recur_dt == mybir.dt.bfloat16
    nc = _new_nc(2)
    wp_d = nc.dram_tensor("wpack", [128, NK * G4], recur_dt, kind="ExternalInput").ap()
    if fuse_l1:
        emb_d = nc.dram_tensor("emb", [V, E], F32, kind="ExternalInput").ap()
        sent_d = nc.dram_tensor("sent", [128, 4], I32, kind="ExternalInput").ap()
        wA_d = nc.dram_tensor("wA", [128, 2 * G4], mybir.dt.bfloat16, kind="ExternalInput").ap()
        wB_d = nc.dram_tensor("wB", [E - 256, G4], mybir.dt.bfloat16, kind="ExternalInput").ap()
        wC_d = nc.dram_tensor("wC", [1, G4], mybir.dt.bfloat16, kind="ExternalInput").ap()
    else:
        xp_d = nc.dram_tensor("xproj", [128, steps * NM], F32, kind="ExternalInput").ap()
    h0_d = nc.dram_tensor("h0c", [128, NK], recur_dt, kind="ExternalInput").ap()
    c0_d = nc.dram_tensor("c0c", [128, NK], F32, kind="ExternalInput").ap()
    hT_d = nc.dram_tensor("hT_out", [128, NK * steps], recur_dt, kind="ExternalOutput").ap()

    with tile.TileContext(nc) as tc, ExitStack() as ctx:
        const = ctx.enter_context(tc.tile_pool(name="const", bufs=1))
        state = ctx.enter_context(tc.tile_pool(name="state", bufs=1))
        ew = ctx.enter_context(tc.tile_pool(name="ew", bufs=4))

        ident = const.tile([128, 128], F32)
        make_identity(nc, ident[:])
        wp = const.tile([128, NK * G4], recur_dt)
        nc.sync.dma_start(wp[:], wp_d[:, :])
        xp = const.tile([128, steps * NM], F32)
        if fuse_l1:
            # --- embedding gather + transpose + input projection, on-chip ---
            phase_a = ExitStack()
            pxp = phase_a.enter_context(tc.tile_pool(name="pxp", bufs=2, space="PSUM"))
            ptp = phase_a.enter_context(tc.tile_pool(name="ptp", bufs=1, space="PSUM"))
            ones = const.tile([1, steps], mybir.dt.bfloat16)
            nc.gpsimd.memset(ones[:], 1.0)
            idx = const.tile([128, 4], I32)
            nc.sync.dma_start(idx[:], sent_d[:, :])
            xg = []
            for c in range(4):
                t = const.tile([128, E], F32, tag=f"xg{c}", name=f"xg{c}")
                nc.gpsimd.indirect_dma_start(
                    out=t[:], out_offset=None, in_=emb_d[:, :],
                    in_offset=bass.IndirectOffsetOnAxis(ap=idx[:, c : c + 1], axis=0),
                )
                xg.append(t)
            ecs = [128, 128, E - 256]
            xT = const.tile([128, 3 * steps], mybir.dt.bfloat16)
            for e in range(3):
                e0 = sum(ecs[:e])
                for c in range(4):
                    pt = ptp.tile([128, 128], F32, space="PSUM", tag="pt")
                    nc.tensor.transpose(
                        out=pt[0 : ecs[e], :], in_=xg[c][:, e0 : e0 + ecs[e]],
                        identity=ident[:],
                    )
                    nc.vector.tensor_copy(
                        xT[0 : ecs[e], e * steps + c * 128 : e * steps + (c + 1) * 128],
                        pt[0 : ecs[e], :],
                    )
            wa_sb = const.tile([128, 2 * G4], mybir.dt.bfloat16)
            nc.sync.dma_start(wa_sb[:], wA_d[:, :])
            wb_sb = const.tile([E - 256, G4], mybir.dt.bfloat16)
            nc.sync.dma_start(wb_sb[:], wB_d[:, :])
            wc_sb = const.tile([1, G4], mybir.dt.bfloat16)
            nc.sync.dma_start(wc_sb[:], wC_d[:, :])
            xpv = xp[:].rearrange("p (t m) -> p t m", m=NM)  # [128, steps, NM]
            for m in range(NM):
                px = pxp.tile([128, steps], F32, space="PSUM", tag="px")
                ms = slice(m * 128, (m + 1) * 128)
                nc.tensor.matmul(px[:], wa_sb[:, ms], xT[0:128, 0:steps],
                                 start=True, stop=False)
                nc.tensor.matmul(px[:], wa_sb[:, G4 + m * 128 : G4 + (m + 1) * 128],
                                 xT[0:128, steps : 2 * steps], start=False, stop=False)
                nc.tensor.matmul(px[:], wb_sb[0 : E - 256, ms],
                                 xT[0 : E - 256, 2 * steps : 3 * steps],
                                 start=False, stop=False)
                nc.tensor.matmul(px[:], wc_sb[0:1, ms], ones[0:1, :],
                                 start=False, stop=True)
                # alternate evacuation between DVE and ScalarE so the copies
                # overlap each other
                if m % 2 == 0:
                    nc.vector.tensor_copy(xpv[:, :, m], px[:])
                else:
                    nc.scalar.copy(xpv[:, :, m], px[:])
            phase_a.close()
        else:
            nc.sync.dma_start(xp[:], xp_d[:, :])
        h0c = const.tile([128, NK], recur_dt)
        nc.sync.dma_start(h0c[:], h0_d[:, :])

        # gate psum pool opens after the phase-A psum pools are closed so the
        # 4 gate tags x 2 bufs can claim all 8 banks
        psum = ctx.enter_context(tc.tile_pool(name="psum", bufs=2, space="PSUM"))

        c_sb = state.tile([128, NK], F32)
        nc.sync.dma_start(c_sb[:], c0_d[:, :])
        hT = state.tile([128, NK * steps], recur_dt)
        hTv = hT[:].rearrange("p (j t) -> p t j", j=NK)  # [128, steps, NK]
        hb16 = state.tile([128, NK], recur_dt, name="hb16") if bf else None

        def step(t, h_cols):
            # Three PSUM banks (i/f, g, o) so each activation starts as soon
            # as its own matmuls finish. PE order if -> g -> o: sigmoid(i,f),
            # tanh(g) and the whole c-update run while the o matmuls stream,
            # leaving only sigmoid(o) + the h-multiply on the exposed path.
            pgif = psum.tile([128, 8], F32, space="PSUM", tag="pgif")
            pgg = psum.tile([128, NK], F32, space="PSUM", tag="pgg")
            pgo = psum.tile([128, NK], F32, space="PSUM", tag="pgo")
            if isinstance(t, int):
                xs_if = xp[:, t * NM : t * NM + 8]
                xs_o = xp[:, t * NM + 8 : t * NM + 12]
                xs_g = xp[:, t * NM + 12 : (t + 1) * NM]
            else:
                xs_if = xp[:, bass.ds(t * NM, 8)]
                xs_o = xp[:, bass.ds(t * NM + 8, NK)]
                xs_g = xp[:, bass.ds(t * NM + 12, NK)]
            skip_mm = "mm" in _skip
            nc.tensor.matmul(pgif[:], ident[:], xs_if, start=True, stop=skip_mm)
            nc.tensor.matmul(pgg[:], ident[:], xs_g, start=True, stop=skip_mm)
            nc.tensor.matmul(pgo[:], ident[:], xs_o, start=True, stop=skip_mm)

            def mms(ms, tile_, last):
                for co, m in enumerate(ms):
                    for j in range(NK):
                        nc.tensor.matmul(
                            tile_[:, co : co + 1],
                            wp[:, j * G4 + m * 128 : j * G4 + (m + 1) * 128],
                            h_cols[j],
                            start=False,
                            stop=(j == NK - 1 and co == len(ms) - 1 and last),
                        )

            gsb = ew.tile([128, NM], F32, tag="gsb")
            if isinstance(t, int):
                hdst = hTv[:, t : t + 1, :]
            else:
                hdst = hTv[:, bass.ds(t, 1), :]
            hdst = hdst.rearrange("p a j -> p (a j)")
            if "elem" in _skip:
                if not skip_mm:
                    mms(range(0, 8), pgif, True)
                    mms(range(12, 16), pgg, True)
                    mms(range(8, 12), pgo, True)
                nc.scalar.activation(hdst, pgif[:, 0:4], AF.Sigmoid)
                if bf:
                    nc.vector.tensor_copy(hb16[:], hdst)
                return
            if not skip_mm:
                mms(range(0, 8), pgif, True)                              # i,f
            nc.scalar.activation(gsb[:, 0:8], pgif[:], AF.Sigmoid)       # sig(i,f)
            t2 = ew.tile([128, NK], F32, tag="t2")
            nc.vector.tensor_mul(t2[:], gsb[:, 4:8], c_sb[:])            # f*c
            if not skip_mm:
                mms(range(12, 16), pgg, True)                             # g
            nc.scalar.activation(gsb[:, 12:16], pgg[:], AF.Tanh)         # tanh(g)
            t1 = ew.tile([128, NK], F32, tag="t1")
            nc.vector.tensor_mul(t1[:], gsb[:, 0:4], gsb[:, 12:16])      # i*g~
            nc.vector.tensor_add(c_sb[:], t1[:], t2[:])                  # c'
            tcc = ew.tile([128, NK], F32, tag="tcc")
            nc.scalar.activation(tcc[:], c_sb[:], AF.Tanh)               # tanh(c')
            if not skip_mm:
                mms(range(8, 12), pgo, True)                              # o
            nc.scalar.activation(gsb[:, 8:12], pgo[:], AF.Sigmoid)       # sig(o)
            if bf:
                # bf16 h feeds the next matvec (critical); fp32 history copy
                # runs off the critical path.
                nc.vector.tensor_mul(hb16[:], gsb[:, 8:12], tcc[:])
                nc.vector.tensor_mul(hdst, gsb[:, 8:12], tcc[:])
            else:
                nc.vector.tensor_mul(hdst, gsb[:, 8:12], tcc[:])         # h = o*tanh(c')

        # t = 0 peeled (h_{-1} = h0)
        step(0, [h0c[:, j : j + 1] for j in range(NK)])

        def body(iv):
            if bf:
                h_cols = [hb16[:, j : j + 1] for j in range(NK)]
            else:
                tm1 = iv - 1
                h_cols = [hT[:, bass.ds(j * steps + tm1, 1)] for j in range(NK)]
            step(iv, h_cols)

        if steps > 1:
            tc.For_i_unrolled_general(
                start=1, end=steps, step=1,
                unrollable_body=lambda iv0, n: [body(iv0 + i) for i in range(n)],
                max_unroll=unroll,
                hint_engines=(mybir.EngineType.PE, mybir.EngineType.Activation,
                              mybir.EngineType.DVE),
            )

        nc.sync.dma_start(hT_d[:, :], hT[:])
    nc.compile()
    return nc


# --------------------------------------------------------------------------
# L3: feats + CRF viterbi + backtrace
# --------------------------------------------------------------------------
def build_l3(steps=L, _skip=()):
    nc = _new_nc(1)
    hcat_d = nc.dram_tensor("hcat", [128, 8 * steps], mybir.dt.bfloat16, kind="ExternalInput").ap()
    wo_d = nc.dram_tensor("woutp", [128, 8 * NT], mybir.dt.bfloat16, kind="ExternalInput").ap()
    bo_d = nc.dram_tensor("bout", [1, NT], mybir.dt.bfloat16, kind="ExternalInput").ap()
    tr_d = nc.dram_tensor("transTp", [32, 32], F32, kind="ExternalInput").ap()
    fv_d = nc.dram_tensor("fvinit", [32, 1], F32, kind="ExternalInput").ap()
    path_d = nc.dram_tensor("path", [1, steps], I32, kind="ExternalOutput").ap()

    with tile.TileContext(nc) as tc, ExitStack() as ctx:
        const = ctx.enter_context(tc.tile_pool(name="const", bufs=1))
        st = ctx.enter_context(tc.tile_pool(name="st", bufs=1))
        psum = ctx.enter_context(tc.tile_pool(name="psum", bufs=2, space="PSUM"))

        hcat = const.tile([128, 8 * steps], mybir.dt.bfloat16)
        nc.sync.dma_start(hcat[:], hcat_d[:, :])
        wo = const.tile([128, 8 * NT], mybir.dt.bfloat16)
        nc.sync.dma_start(wo[:], wo_d[:, :])
        bo = const.tile([1, NT], mybir.dt.bfloat16)
        nc.sync.dma_start(bo[:], bo_d[:, :])
        trT = const.tile([32, 32], F32)
        nc.sync.dma_start(trT[:], tr_d[:, :])
        fvi = const.tile([32, 1], F32)
        nc.sync.dma_start(fvi[:], fv_d[:, :])
        ones = const.tile([1, max(steps, NT)], mybir.dt.bfloat16)
        nc.gpsimd.memset(ones[:], 1.0)

        # feats^T [20, steps]
        pf = psum.tile([32, steps], F32, space="PSUM", tag="pf")
        for j in range(8):
            nc.tensor.matmul(
                pf[0:NT, :], wo[:, j * NT : (j + 1) * NT],
                hcat[:, j * steps : (j + 1) * steps],
                start=(j == 0), stop=False,
            )
        nc.tensor.matmul(pf[0:NT, :], bo[0:1, :], ones[0:1, 0:steps], start=False, stop=True)
        feats = st.tile([32, steps], F32)
        nc.gpsimd.memset(feats[:], 0.0)
        nc.scalar.activation(feats[0:NT, :], pf[0:NT, :], AF.Copy)

        # CRF forward
        scT = st.tile([32, 32], F32)   # scores^T[prev, next]
        nc.gpsimd.memset(scT[:], 0.0)
        bpt = st.tile([32, 8 * steps], U32)  # top8 indices per step

        # Keep all transposed score tiles: max_index is not on the fv
        # dependency chain, so it is deferred and batched after the loop
        # (3 serial DVE ops per step instead of 4).
        schist = st.tile([32, 32 * steps], F32)
        mxhist = st.tile([32, 8 * steps], F32)
        nc.gpsimd.memset(mxhist[:], 0.0)
        nc.vector.tensor_scalar_add(scT[:, 0:NT], trT[:, 0:NT], fvi[:, 0:1])
        crf_steps = 1 if "crf" in _skip else steps
        mx = None
        for t in range(crf_steps):
            sct = schist[:, 32 * t : 32 * (t + 1)]
            nc.vector.transpose(sct, scT[:])
            mx = mxhist[:, 8 * t : 8 * t + 8]
            nc.vector.max(mx[0:NT, :], sct[0:NT, 0:NT])
            if t < steps - 1:
                nc.vector.scalar_tensor_tensor(
                    out=scT[:, 0:NT],
                    in0=trT[:, 0:NT],
                    scalar=mx[:, 0:1],
                    in1=feats[:, t : t + 1].to_broadcast([32, NT]),
                    op0=OP.add,
                    op1=OP.add,
                )
        def maxidx_batch(lo, hi):
            for t in range(lo, min(hi, crf_steps)):
                nc.vector.max_index(
                    bpt[0:NT, 8 * t : 8 * t + 8],
                    mxhist[0:NT, 8 * t : 8 * t + 8],
                    schist[0:NT, 32 * t : 32 * t + NT],
                )
        # terminal[p] = fv_raw[p] + feats[last, p] + trans[STOP, p]
        term = st.tile([32, 1], F32)
        nc.gpsimd.memset(term[:], NEG)
        nc.vector.scalar_tensor_tensor(
            out=term[0:NT, :],
            in0=trT[0:NT, STOP : STOP + 1],
            scalar=mx[0:NT, 0:1],
            in1=feats[0:NT, steps - 1 : steps],
            op0=OP.add,
            op1=OP.add,
        )
        # best tag one-hot
        t32 = st.tile([32, 32], F32)
        nc.gpsimd.memset(t32[:], NEG)
        nc.vector.tensor_copy(t32[:, 0:1], term[:])
        tT = st.tile([32, 32], F32)
        nc.vector.transpose(tT[:], t32[:])
        mxt = st.tile([32, 8], F32)
        nc.vector.max(mxt[0:1, :], tT[0:1, 0:NT])
        onesf = st.tile([1, NT], F32)
        nc.gpsimd.memset(onesf[:], 1.0)
        pmx = psum.tile([32, 1], F32, space="PSUM", tag="pmx")
        nc.tensor.matmul(pmx[0:NT, :], onesf[0:1, 0:NT], mxt[0:1, 0:1], start=True, stop=True)
        mxb = st.tile([32, 1], F32)
        nc.vector.tensor_copy(mxb[0:NT, :], pmx[0:NT, :])
        pathOH = st.tile([32, steps], F32)
        nc.gpsimd.memset(pathOH[:], 0.0)
        nc.vector.tensor_scalar(
            pathOH[0:NT, steps - 1 : steps], term[0:NT, :], mxb[0:NT, 0:1], None,
            OP.is_equal,
        )

        # one-hot backpointer matrices M_all[p, t*20+n] = (bptr[p,t] == n),
        # built in half-chunks so the low half's argmax/one-hot work hides
        # under the high half's backtrace chain.
        iotar = st.tile([32, NT], I32)
        nc.gpsimd.iota(iotar[:], pattern=[[1, NT]], base=0, channel_multiplier=0)
        iotarf = st.tile([32, NT], F32)
        nc.vector.tensor_copy(iotarf[:], iotar[:])
        bpf = st.tile([32, steps], F32)
        mall = st.tile([32, steps * NT], F32)

        def mall_chunk(lo, hi):
            n = hi - lo
            nc.vector.tensor_copy(
                bpf[0:NT, lo:hi],
                bpt[0:NT, 8 * lo : 8 * hi].rearrange("p (t e) -> p t e", e=8)[:, :, 0],
            )
            nc.vector.tensor_tensor(
                out=mall[0:NT, lo * NT : hi * NT].rearrange("p (t n) -> p t n", n=NT),
                in0=bpf[0:NT, lo:hi].rearrange("p (t o) -> p t o", o=1)
                    .broadcast_to([NT, n, NT]),
                in1=iotarf[0:NT, :].rearrange("p (o n) -> p o n", o=1)
                    .broadcast_to([NT, n, NT]),
                op=OP.is_equal,
            )

        def bt_chain(lo, hi, filler=None):
            if "backtrace" in _skip:
                return
            for t in range(hi - 2, lo - 2, -1):
                if t < 0:
                    break
                pv = psum.tile([32, 1], F32, space="PSUM", tag="pv")
                nc.tensor.matmul(
                    pv[0:NT, :],
                    mall[0:NT, (t + 1) * NT : (t + 2) * NT],
                    pathOH[0:NT, t + 1 : t + 2],
                    start=True, stop=True,
                )
                # ScalarE copy keeps the DVE free for the interleaved argmaxes
                nc.scalar.copy(pathOH[0:NT, t : t + 1], pv[0:NT, :])
                if filler is not None:
                    next(filler, None)

        def maxidx_gen(lo, hi):
            # one deferred argmax per yield, interleaved between chain links
            for t in range(lo, min(hi, crf_steps)):
                nc.vector.max_index(
                    bpt[0:NT, 8 * t : 8 * t + 8],
                    mxhist[0:NT, 8 * t : 8 * t + 8],
                    schist[0:NT, 32 * t : 32 * t + NT],
                )
                yield t

        half = steps // 2
        maxidx_batch(half, steps)
        mall_chunk(half, steps)
        bt_chain(half, steps, filler=maxidx_gen(0, half))
        mall_chunk(0, half)
        bt_chain(0, half)

        # path_int[t] = iota . pathOH[:, t]
        iotac = st.tile([32, 1], I32)
        nc.gpsimd.iota(iotac[:], pattern=[[0, 1]], base=0, channel_multiplier=1)
        iotacf = st.tile([32, 1], F32)
        nc.vector.tensor_copy(iotacf[:], iotac[:])
        pp = psum.tile([32, steps], F32, space="PSUM", tag="pp")
        nc.tensor.matmul(pp[0:1, :], iotacf[0:NT, :], pathOH[0:NT, :], start=True, stop=True)
        path_sb = st.tile([1, steps], I32)
        nc.vector.tensor_copy(path_sb[:], pp[0:1, :])
        nc.sync.dma_start(path_d[:, :], path_sb[:])
    nc.compile()
    return nc


# --------------------------------------------------------------------------
# host glue
# --------------------------------------------------------------------------
def _prep_l1_inputs(sentence, embed_table, wih, bih, bhh):
    sent = np.ascontiguousarray(
        np.asarray(sentence, np.int32).reshape(4, 128).T
    )
    ins = {"emb": np.asarray(embed_table, np.float32), "sent": sent}
    for d in ("f", "b"):
        w = np.asarray(wih[d], np.float32)[_PERM]          # [2048, 300]
        b = (np.asarray(bih[d], np.float32) + np.asarray(bhh[d], np.float32))[_PERM]
        wT = np.ascontiguousarray(w.T)                     # [300, 2048]
        ins[f"wA_{d}"] = np.ascontiguousarray(
            np.concatenate([wT[0:128], wT[128:256]], axis=1)
        )
        ins[f"wB_{d}"] = np.ascontiguousarray(wT[256:300])
        ins[f"wC_{d}"] = np.ascontiguousarray(b[None, :])
    return ins


def _prep_l2_inputs(xprojT, whh, h0, c0):
    # xprojT: [2048, 512] (gate-permuted rows, bias included)
    import ml_dtypes
    rdt = np.float32 if RECUR_DT == F32 else ml_dtypes.bfloat16
    w = np.asarray(whh, np.float32)[_PERM]                 # [2048, 512]
    wT = np.ascontiguousarray(w.T)                         # [512, 2048]
    wpack = np.ascontiguousarray(
        wT.reshape(NK, 128, G4).transpose(1, 0, 2).reshape(128, NK * G4)
    ).astype(rdt)
    xp = np.ascontiguousarray(
        xprojT.reshape(NM, 128, L).transpose(1, 2, 0).reshape(128, L * NM)
    )
    h0c = np.ascontiguousarray(
        np.asarray(h0, np.float32).reshape(NK, 128).T
    ).astype(rdt)
    c0c = np.ascontiguousarray(np.asarray(c0, np.float32).reshape(NK, 128).T)
    return {"wpack": wpack, "xproj": xp, "h0c": h0c, "c0c": c0c}


def _prep_l3_inputs(hTf, hTb_scan, wout, bout, transitions):
    # hTf / hTb_scan: [128, 4*512]; backward scan is in scan order (reversed time)
    blocks = [hTf[:, j * L : (j + 1) * L] for j in range(NK)]
    blocks += [hTb_scan[:, j * L : (j + 1) * L][:, ::-1] for j in range(NK)]
    hcat = np.ascontiguousarray(np.concatenate(blocks, axis=1))
    woT = np.ascontiguousarray(np.asarray(wout, np.float32).T)  # [1024, 20]
    wop = np.ascontiguousarray(
        np.concatenate([woT[j * 128 : (j + 1) * 128] for j in range(8)], axis=1)
    )
    trTp = np.zeros((32, 32), np.float32)
    trTp[0:NT, 0:NT] = np.asarray(transitions, np.float32).T
    fvi = np.zeros((32, 1), np.float32)
    fvi[0:NT, 0] = NEG
    fvi[START, 0] = 0.0
    import ml_dtypes
    return {
        "hcat": hcat.astype(ml_dtypes.bfloat16),
        "woutp": wop.astype(ml_dtypes.bfloat16),
        "bout": np.ascontiguousarray(
            np.asarray(bout, np.float32)[None, :]).astype(ml_dtypes.bfloat16),
        "transTp": trTp,
        "fvinit": fvi,
    }


def _get(name, builder):
    if name not in _CACHE:
        _CACHE[name] = builder()
    return _CACHE[name]


def _prep_l12_inputs(sentence, embed_table, wih, bih, bhh, whh, h0, c0, reverse):
    import ml_dtypes
    rdt = np.float32 if RECUR_DT == F32 else ml_dtypes.bfloat16
    s = np.asarray(sentence, np.int32)
    if reverse:
        s = s[::-1]
    ins = {
        "emb": np.asarray(embed_table, np.float32),
        "sent": np.ascontiguousarray(s.reshape(4, 128).T),
    }
    w = np.asarray(wih, np.float32)[_PERM]                 # [2048, 300]
    b = (np.asarray(bih, np.float32) + np.asarray(bhh, np.float32))[_PERM]
    wT = np.ascontiguousarray(w.T)                         # [300, 2048]
    ins["wA"] = np.ascontiguousarray(
        np.concatenate([wT[0:128], wT[128:256]], axis=1)).astype(ml_dtypes.bfloat16)
    ins["wB"] = np.ascontiguousarray(wT[256:300]).astype(ml_dtypes.bfloat16)
    ins["wC"] = np.ascontiguousarray(b[None, :]).astype(ml_dtypes.bfloat16)
    wh = np.asarray(whh, np.float32)[_PERM]                # [2048, 512]
    whT = np.ascontiguousarray(wh.T)                       # [512, 2048]
    ins["wpack"] = np.ascontiguousarray(
        whT.reshape(NK, 128, G4).transpose(1, 0, 2).reshape(128, NK * G4)
    ).astype(rdt)
    ins["h0c"] = np.ascontiguousarray(
        np.asarray(h0, np.float32).reshape(NK, 128).T
    ).astype(rdt)
    ins["c0c"] = np.ascontiguousarray(np.asarray(c0, np.float32).reshape(NK, 128).T)
    return ins


def kernel(sentence, embed_table, w_ih_f, w_hh_f, b_ih_f, b_hh_f,
           w_ih_b, w_hh_b, b_ih_b, b_hh_b, h0, c0, w_out, b_out, transitions):
    h0 = np.asarray(h0, np.float32)
    c0 = np.asarray(c0, np.float32)

    # ---- L12: per-core gather + input projection + LSTM recurrence
    nc2 = _get("l12", build_l2)
    in_f = _prep_l12_inputs(sentence, embed_table, w_ih_f, b_ih_f, b_hh_f,
                            w_hh_f, h0[0], c0[0], reverse=False)
    in_b = _prep_l12_inputs(sentence, embed_table, w_ih_b, b_ih_b, b_hh_b,
                            w_hh_b, h0[1], c0[1], reverse=True)
    r2 = run_bass_kernel_spmd(nc2, [in_f, in_b], core_ids=[0, 1]).results
    hTf = r2[0]["hT_out"]       # [128, 2048]
    hTb_scan = r2[1]["hT_out"]  # backward scan order

    # ---- L3: feats + viterbi + backtrace
    nc3 = _get("l3", build_l3)
    ins3 = _prep_l3_inputs(hTf, hTb_scan, w_out, b_out, transitions)
    r3 = run_bass_kernel_spmd(nc3, [ins3], core_ids=[0]).results[0]
    return np.ascontiguousarray(r3["path"].reshape(L)).astype(np.int32)



# revision 2
# speedup vs baseline: 1.4153x; 1.4153x over previous
"""BiLSTM-CRF Trainium2 kernel (Bass/Tile), two launches.

Strategy (batch=1, L=512, sequential recurrence is the critical path):
  L12 (2 cores, SPMD): one LSTM direction per core; the backward core
      simply receives a host-reversed sentence. Each core does its own
      embedding gather (indirect DMA over the full table), PE transposes,
      bf16 input projection x@Wih^T (+bias folded in via a ones-row matmul;
      fp32 PSUM accumulation),
      then the 512-step recurrence. Per step, h@Whh^T runs as 64
      weight-stationary bf16 matmuls (gates land [128,16] across two PSUM
      banks; g-gates in their own bank so tanh(g) starts early), i/f/o
      sigmoid + c/h update on ACT/DVE; h is produced in bf16 for the next
      matvec with an fp32 history copy off the critical path. bf16 for the
      recurrent matvec reproduces the exact fp32 Viterbi path on the
      reference inputs (verified end-to-end; set RECUR_DT = F32 to fall
      back to full fp32).
  L3 (1 core): feats = [hf,hb]@Wout^T + b on PE; CRF Viterbi forward scan
      (3 serial DVE ops/step: score-update, 32x32 transpose, max; argmax
      extraction deferred and batched off the dependency chain); backtrace
      as a one-hot matmul chain on PE with ScalarE PSUM evacuation.

Host work is limited to sharding glue: dtype casts, weight re-layout,
time reversal for the backward direction, and final unshard/reshape.
"""

import numpy as np
from contextlib import ExitStack

import concourse.bass as bass
import concourse.tile as tile
from concourse import bacc, mybir
from concourse.bass_utils import run_bass_kernel_spmd
from concourse.masks import make_identity

F32 = mybir.dt.float32
I32 = mybir.dt.int32
U32 = mybir.dt.uint32
AF = mybir.ActivationFunctionType
OP = mybir.AluOpType

V, E, H, L = 100000, 300, 512, 512
NT, START, STOP, NEG = 20, 18, 19, -10000.0
G4 = 4 * H  # 2048
NM = G4 // 128  # 16 gate column-chunks
NK = H // 128   # 4 h row-chunks

# gate row order used on-chip: i, f, o, g (so sigmoid covers cols 0:12)
_PERM = np.concatenate([
    np.arange(0, H),          # i
    np.arange(H, 2 * H),      # f
    np.arange(3 * H, 4 * H),  # o
    np.arange(2 * H, 3 * H),  # g
])

_CACHE: dict = {}

# bf16 for the recurrent matvec (weights + h): halves the PE weight-load
# bottleneck. Verified to reproduce the exact fp32 Viterbi path on the
# reference inputs. Set to F32 to fall back to full fp32.
RECUR_DT = mybir.dt.bfloat16


def _new_nc(num_devices):
    return bacc.Bacc(
        "TRN2", target_bir_lowering=False, debug=False, num_devices=num_devices
    )


# --------------------------------------------------------------------------
# L1: gather + input projection
# --------------------------------------------------------------------------
def build_l1():
    nc = _new_nc(1)
    emb = nc.dram_tensor("emb", [V, E], F32, kind="ExternalInput").ap()
    sent = nc.dram_tensor("sent", [128, 4], I32, kind="ExternalInput").ap()
    wA = {}
    wB = {}
    wC = {}
    xout = {}
    for d in ("f", "b"):
        wA[d] = nc.dram_tensor(f"wA_{d}", [128, 2 * G4], F32, kind="ExternalInput").ap()
        wB[d] = nc.dram_tensor(f"wB_{d}", [E - 256, G4], F32, kind="ExternalInput").ap()
        wC[d] = nc.dram_tensor(f"wC_{d}", [1, G4], F32, kind="ExternalInput").ap()
        xout[d] = nc.dram_tensor(f"xout_{d}", [G4, L], F32, kind="ExternalOutput").ap()

    with tile.TileContext(nc) as tc, ExitStack() as ctx:
        const = ctx.enter_context(tc.tile_pool(name="const", bufs=1))
        work = ctx.enter_context(tc.tile_pool(name="work", bufs=2))
        psum = ctx.enter_context(tc.tile_pool(name="psum", bufs=2, space="PSUM"))
        pxp = ctx.enter_context(tc.tile_pool(name="pxp", bufs=4, space="PSUM"))

        ident = const.tile([128, 128], F32)
        make_identity(nc, ident[:])
        ones = const.tile([1, L], F32)
        nc.gpsimd.memset(ones[:], 1.0)

        idx = const.tile([128, 4], I32)
        nc.sync.dma_start(idx[:], sent[:, :])

        # gather x rows: 4 chunks of 128 sentence positions
        xg = []
        for c in range(4):
            t = const.tile([128, E], F32, tag=f"xg{c}", name=f"xg{c}")
            nc.gpsimd.indirect_dma_start(
                out=t[:],
                out_offset=None,
                in_=emb[:, :],
                in_offset=bass.IndirectOffsetOnAxis(ap=idx[:, c : c + 1], axis=0),
            )
            xg.append(t)

        # transpose x -> xT [300(3 chunks), 512]; chunk e occupies cols e*512..
        ecs = [128, 128, E - 256]
        xT = const.tile([128, 3 * L], F32)
        for e in range(3):
            e0 = sum(ecs[:e])
            for c in range(4):
                pt = psum.tile([128, 128], F32, space="PSUM", tag="pt")
                nc.tensor.transpose(
                    out=pt[0 : ecs[e], :], in_=xg[c][:, e0 : e0 + ecs[e]], identity=ident[:]
                )
                nc.vector.tensor_copy(
                    xT[0 : ecs[e], e * L + c * 128 : e * L + (c + 1) * 128],
                    pt[0 : ecs[e], :],
                )

        # load weights to SBUF
        wa_sb, wb_sb, wc_sb = {}, {}, {}
        for d in ("f", "b"):
            wa_sb[d] = const.tile([128, 2 * G4], F32, tag=f"wa{d}", name=f"wa{d}")
            nc.sync.dma_start(wa_sb[d][:], wA[d][:, :])
            wb_sb[d] = const.tile([E - 256, G4], F32, tag=f"wb{d}", name=f"wb{d}")
            nc.sync.dma_start(wb_sb[d][:], wB[d][:, :])
            wc_sb[d] = const.tile([1, G4], F32, tag=f"wc{d}", name=f"wc{d}")
            nc.sync.dma_start(wc_sb[d][:], wC[d][:, :])

        # xprojT[g, t] = sum_e WihT[e, g] * xT[e, t]  (+ bias via ones row)
        for d in ("f", "b"):
            for m in range(NM):
                px = pxp.tile([128, L], F32, space="PSUM", tag="px")
                ms = slice(m * 128, (m + 1) * 128)
                nc.tensor.matmul(
                    px[:], wa_sb[d][:, m * 128 : (m + 1) * 128], xT[0:128, 0:L],
                    start=True, stop=False,
                )
                nc.tensor.matmul(
                    px[:], wa_sb[d][:, G4 + m * 128 : G4 + (m + 1) * 128],
                    xT[0:128, L : 2 * L], start=False, stop=False,
                )
                nc.tensor.matmul(
                    px[:], wb_sb[d][0 : E - 256, ms], xT[0 : E - 256, 2 * L : 3 * L],
                    start=False, stop=False,
                )
                nc.tensor.matmul(
                    px[:], wc_sb[d][0:1, ms], ones[0:1, :], start=False, stop=True,
                )
                sb = work.tile([128, L], F32, tag="xps")
                nc.vector.tensor_copy(sb[:], px[:])
                nc.sync.dma_start(xout[d][ms, :], sb[:])
    nc.compile()
    return nc


# --------------------------------------------------------------------------
# L2: one LSTM direction (SPMD over 2 cores)
# --------------------------------------------------------------------------
def build_l2(steps=L, unroll=48, recur_dt=None, _skip=(), fuse_l1=True):
    recur_dt = recur_dt if recur_dt is not None else RECUR_DT
    bf = recur_dt == mybir.dt.bfloat16
    nc = _new_nc(2)
    wp_d = nc.dram_tensor("wpack", [128, NK * G4], recur_dt, kind="ExternalInput").ap()
    if fuse_l1:
        emb_d = nc.dram_tensor("emb", [V, E], F32, kind="ExternalInput").ap()
        sent_d = nc.dram_tensor("sent", [128, 4], I32, kind="ExternalInput").ap()
        wA_d = nc.dram_tensor("wA", [128, 2 * G4], mybir.dt.bfloat16, kind="ExternalInput").ap()
        wB_d = nc.dram_tensor("wB", [E - 256, G4], mybir.dt.bfloat16, kind="ExternalInput").ap()
        wC_d = nc.dram_tensor("wC", [1, G4], mybir.dt.bfloat16, kind="ExternalInput").ap()
    else:
        xp_d = nc.dram_tensor("xproj", [128, steps * NM], F32, kind="ExternalInput").ap()
    h0_d = nc.dram_tensor("h0c", [128, NK], recur_dt, kind="ExternalInput").ap()
    c0_d = nc.dram_tensor("c0c", [128, NK], F32, kind="ExternalInput").ap()
    hT_d = nc.dram_tensor("hT_out", [128, NK * steps], recur_dt, kind="ExternalOutput").ap()

    with tile.TileContext(nc) as tc, ExitStack() as ctx:
        const = ctx.enter_context(tc.tile_pool(name="const", bufs=1))
        state = ctx.enter_context(tc.tile_pool(name="state", bufs=1))
        ew = ctx.enter_context(tc.tile_pool(name="ew", bufs=4))

        ident = const.tile([128, 128], F32)
        make_identity(nc, ident[:])
        wp = const.tile([128, NK * G4], recur_dt)
        nc.sync.dma_start(wp[:], wp_d[:, :])
        xp = const.tile([128, steps * NM], F32)
        if fuse_l1:
            # --- embedding gather + transpose + input projection, on-chip ---
            phase_a = ExitStack()
            pxp = phase_a.enter_context(tc.tile_pool(name="pxp", bufs=2, space="PSUM"))
            ptp = phase_a.enter_context(tc.tile_pool(name="ptp", bufs=1, space="PSUM"))
            ones = const.tile([1, steps], mybir.dt.bfloat16)
            nc.gpsimd.memset(ones[:], 1.0)
            idx = const.tile([128, 4], I32)
            nc.sync.dma_start(idx[:], sent_d[:, :])
            xg = []
            for c in range(4):
                t = const.tile([128, E], F32, tag=f"xg{c}", name=f"xg{c}")
                nc.gpsimd.indirect_dma_start(
                    out=t[:], out_offset=None, in_=emb_d[:, :],
                    in_offset=bass.IndirectOffsetOnAxis(ap=idx[:, c : c + 1], axis=0),
                )
                xg.append(t)
            ecs = [128, 128, E - 256]
            xT = const.tile([128, 3 * steps], mybir.dt.bfloat16)
            for e in range(3):
                e0 = sum(ecs[:e])
                for c in range(4):
                    pt = ptp.tile([128, 128], F32, space="PSUM", tag="pt")
                    nc.tensor.transpose(
                        out=pt[0 : ecs[e], :], in_=xg[c][:, e0 : e0 + ecs[e]],
                        identity=ident[:],
                    )
                    nc.vector.tensor_copy(
                        xT[0 : ecs[e], e * steps + c * 128 : e * steps + (c + 1) * 128],
                        pt[0 : ecs[e], :],
                    )
            wa_sb = const.tile([128, 2 * G4], mybir.dt.bfloat16)
            nc.sync.dma_start(wa_sb[:], wA_d[:, :])
            wb_sb = const.tile([E - 256, G4], mybir.dt.bfloat16)
            nc.sync.dma_start(wb_sb[:], wB_d[:, :])
            wc_sb = const.tile([1, G4], mybir.dt.bfloat16)
            nc.sync.dma_start(wc_sb[:], wC_d[:, :])
            xpv = xp[:].rearrange("p (t m) -> p t m", m=NM)  # [128, steps, NM]
            for m in range(NM):
                px = pxp.tile([128, steps], F32, space="PSUM", tag="px")
                ms = slice(m * 128, (m + 1) * 128)
                nc.tensor.matmul(px[:], wa_sb[:, ms], xT[0:128, 0:steps],
                                 start=True, stop=False)
                nc.tensor.matmul(px[:], wa_sb[:, G4 + m * 128 : G4 + (m + 1) * 128],
                                 xT[0:128, steps : 2 * steps], start=False, stop=False)
                nc.tensor.matmul(px[:], wb_sb[0 : E - 256, ms],
                                 xT[0 : E - 256, 2 * steps : 3 * steps],
                                 start=False, stop=False)
                nc.tensor.matmul(px[:], wc_sb[0:1, ms], ones[0:1, :],
                                 start=False, stop=True)
                # alternate evacuation between DVE and ScalarE so the copies
                # overlap each other
                if m % 2 == 0:
                    nc.vector.tensor_copy(xpv[:, :, m], px[:])
                else:
                    nc.scalar.copy(xpv[:, :, m], px[:])
            phase_a.close()
        else:
            nc.sync.dma_start(xp[:], xp_d[:, :])
        h0c = const.tile([128, NK], recur_dt)
        nc.sync.dma_start(h0c[:], h0_d[:, :])

        # gate psum pool opens after the phase-A psum pools are closed so the
        # 4 gate tags x 2 bufs can claim all 8 banks
        psum = ctx.enter_context(tc.tile_pool(name="psum", bufs=2, space="PSUM"))

        c_sb = state.tile([128, NK], F32)
        nc.sync.dma_start(c_sb[:], c0_d[:, :])
        hT = state.tile([128, NK * steps], recur_dt)
        hTv = hT[:].rearrange("p (j t) -> p t j", j=NK)  # [128, steps, NK]
        hb16 = state.tile([128, NK], recur_dt, name="hb16") if bf else None

        def step(t, h_cols):
            # Three PSUM banks (i/f, g, o) so each activation starts as soon
            # as its own matmuls finish. PE order if -> g -> o: sigmoid(i,f),
            # tanh(g) and the whole c-update run while the o matmuls stream,
            # leaving only sigmoid(o) + the h-multiply on the exposed path.
            pgif = psum.tile([128, 8], F32, space="PSUM", tag="pgif")
            pgg = psum.tile([128, NK], F32, space="PSUM", tag="pgg")
            pgo = psum.tile([128, NK], F32, space="PSUM", tag="pgo")
            if isinstance(t, int):
                xs_if = xp[:, t * NM : t * NM + 8]
                xs_o = xp[:, t * NM + 8 : t * NM + 12]
                xs_g = xp[:, t * NM + 12 : (t + 1) * NM]
            else:
                xs_if = xp[:, bass.ds(t * NM, 8)]
                xs_o = xp[:, bass.ds(t * NM + 8, NK)]
                xs_g = xp[:, bass.ds(t * NM + 12, NK)]
            skip_mm = "mm" in _skip
            nc.tensor.matmul(pgif[:], ident[:], xs_if, start=True, stop=skip_mm)
            nc.tensor.matmul(pgg[:], ident[:], xs_g, start=True, stop=skip_mm)
            nc.tensor.matmul(pgo[:], ident[:], xs_o, start=True, stop=skip_mm)

            def mms(ms, tile_, last):
                for co, m in enumerate(ms):
                    for j in range(NK):
                        nc.tensor.matmul(
                            tile_[:, co : co + 1],
                            wp[:, j * G4 + m * 128 : j * G4 + (m + 1) * 128],
                            h_cols[j],
                            start=False,
                            stop=(j == NK - 1 and co == len(ms) - 1 and last),
                        )

            gsb = ew.tile([128, NM], F32, tag="gsb")
            if isinstance(t, int):
                hdst = hTv[:, t : t + 1, :]
            else:
                hdst = hTv[:, bass.ds(t, 1), :]
            hdst = hdst.rearrange("p a j -> p (a j)")
            if "elem" in _skip:
                if not skip_mm:
                    mms(range(0, 8), pgif, True)
                    mms(range(12, 16), pgg, True)
                    mms(range(8, 12), pgo, True)
                nc.scalar.activation(hdst, pgif[:, 0:4], AF.Sigmoid)
                if bf:
                    nc.vector.tensor_copy(hb16[:], hdst)
                return
            if not skip_mm:
                mms(range(0, 8), pgif, True)                              # i,f
            nc.scalar.activation(gsb[:, 0:8], pgif[:], AF.Sigmoid)       # sig(i,f)
            t2 = ew.tile([128, NK], F32, tag="t2")
            nc.vector.tensor_mul(t2[:], gsb[:, 4:8], c_sb[:])            # f*c
            if not skip_mm:
                mms(range(12, 16), pgg, True)                             # g
            nc.scalar.activation(gsb[:, 12:16], pgg[:], AF.Tanh)         # tanh(g)
            t1 = ew.tile([128, NK], F32, tag="t1")
            nc.vector.tensor_mul(t1[:], gsb[:, 0:4], gsb[:, 12:16])      # i*g~
            nc.vector.tensor_add(c_sb[:], t1[:], t2[:])                  # c'
            tcc = ew.tile([128, NK], F32, tag="tcc")
            if not skip_mm:
                mms(range(8, 12), pgo, True)                              # o
            nc.scalar.activation(gsb[:, 8:12], pgo[:], AF.Sigmoid)       # sig(o)
            nc.scalar.activation(tcc[:], c_sb[:], AF.Tanh)               # tanh(c')
            if bf:
                # bf16 h feeds the next matvec (critical); fp32 history copy
                # runs off the critical path.
                nc.vector.tensor_mul(hb16[:], gsb[:, 8:12], tcc[:])
                nc.vector.tensor_mul(hdst, gsb[:, 8:12], tcc[:])
            else:
                nc.vector.tensor_mul(hdst, gsb[:, 8:12], tcc[:])         # h = o*tanh(c')

        # t = 0 peeled (h_{-1} = h0)
        step(0, [h0c[:, j : j + 1] for j in range(NK)])

        def body(iv):
            if bf:
                h_cols = [hb16[:, j : j + 1] for j in range(NK)]
            else:
                tm1 = iv - 1
                h_cols = [hT[:, bass.ds(j * steps + tm1, 1)] for j in range(NK)]
            step(iv, h_cols)

        if steps > 1:
            tc.For_i_unrolled_general(
                start=1, end=steps, step=1,
                unrollable_body=lambda iv0, n: [body(iv0 + i) for i in range(n)],
                max_unroll=unroll,
                hint_engines=(mybir.EngineType.PE, mybir.EngineType.Activation,
                              mybir.EngineType.DVE),
            )

        nc.sync.dma_start(hT_d[:, :], hT[:])
    nc.compile()
    return nc


# --------------------------------------------------------------------------
# L3: feats + CRF viterbi + backtrace
# --------------------------------------------------------------------------
def build_l3(steps=L, _skip=()):
    nc = _new_nc(1)
    hcat_d = nc.dram_tensor("hcat", [128, 8 * steps], mybir.dt.bfloat16, kind="ExternalInput").ap()
    wo_d = nc.dram_tensor("woutp", [128, 8 * NT], mybir.dt.bfloat16, kind="ExternalInput").ap()
    bo_d = nc.dram_tensor("bout", [1, NT], mybir.dt.bfloat16, kind="ExternalInput").ap()
    tr_d = nc.dram_tensor("transTp", [32, 32], F32, kind="ExternalInput").ap()
    fv_d = nc.dram_tensor("fvinit", [32, 1], F32, kind="ExternalInput").ap()
    path_d = nc.dram_tensor("path", [1, steps], I32, kind="ExternalOutput").ap()

    with tile.TileContext(nc) as tc, ExitStack() as ctx:
        const = ctx.enter_context(tc.tile_pool(name="const", bufs=1))
        st = ctx.enter_context(tc.tile_pool(name="st", bufs=1))
        psum = ctx.enter_context(tc.tile_pool(name="psum", bufs=2, space="PSUM"))

        hcat = const.tile([128, 8 * steps], mybir.dt.bfloat16)
        nc.sync.dma_start(hcat[:], hcat_d[:, :])
        wo = const.tile([128, 8 * NT], mybir.dt.bfloat16)
        nc.sync.dma_start(wo[:], wo_d[:, :])
        bo = const.tile([1, NT], mybir.dt.bfloat16)
        nc.sync.dma_start(bo[:], bo_d[:, :])
        trT = const.tile([32, 32], F32)
        nc.sync.dma_start(trT[:], tr_d[:, :])
        fvi = const.tile([32, 1], F32)
        nc.sync.dma_start(fvi[:], fv_d[:, :])
        ones = const.tile([1, max(steps, NT)], mybir.dt.bfloat16)
        nc.gpsimd.memset(ones[:], 1.0)

        # feats^T [20, steps]
        pf = psum.tile([32, steps], F32, space="PSUM", tag="pf")
        for j in range(8):
            nc.tensor.matmul(
                pf[0:NT, :], wo[:, j * NT : (j + 1) * NT],
                hcat[:, j * steps : (j + 1) * steps],
                start=(j == 0), stop=False,
            )
        nc.tensor.matmul(pf[0:NT, :], bo[0:1, :], ones[0:1, 0:steps], start=False, stop=True)
        feats = st.tile([32, steps], F32)
        nc.gpsimd.memset(feats[:], 0.0)
        nc.scalar.activation(feats[0:NT, :], pf[0:NT, :], AF.Copy)

        # CRF forward
        scT = st.tile([32, 32], F32)   # scores^T[prev, next]
        nc.gpsimd.memset(scT[:], 0.0)
        bpt = st.tile([32, 8 * steps], U32)  # top8 indices per step

        # Keep all transposed score tiles: max_index is not on the fv
        # dependency chain, so it is deferred and batched after the loop
        # (3 serial DVE ops per step instead of 4).
        schist = st.tile([32, 32 * steps], F32)
        mxhist = st.tile([32, 8 * steps], F32)
        nc.gpsimd.memset(mxhist[:], 0.0)
        nc.vector.tensor_scalar_add(scT[:, 0:NT], trT[:, 0:NT], fvi[:, 0:1])
        crf_steps = 1 if "crf" in _skip else steps
        mx = None
        for t in range(crf_steps):
            sct = schist[:, 32 * t : 32 * (t + 1)]
            nc.vector.transpose(sct, scT[:])
            mx = mxhist[:, 8 * t : 8 * t + 8]
            nc.vector.max(mx[0:NT, :], sct[0:NT, 0:NT])
            if t < steps - 1:
                nc.vector.scalar_tensor_tensor(
                    out=scT[:, 0:NT],
                    in0=trT[:, 0:NT],
                    scalar=mx[:, 0:1],
                    in1=feats[:, t : t + 1].to_broadcast([32, NT]),
                    op0=OP.add,
                    op1=OP.add,
                )
        def maxidx_batch(lo, hi):
            for t in range(lo, min(hi, crf_steps)):
                nc.vector.max_index(
                    bpt[0:NT, 8 * t : 8 * t + 8],
                    mxhist[0:NT, 8 * t : 8 * t + 8],
                    schist[0:NT, 32 * t : 32 * t + NT],
                )
        # terminal[p] = fv_raw[p] + feats[last, p] + trans[STOP, p]
        term = st.tile([32, 1], F32)
        nc.gpsimd.memset(term[:], NEG)
        nc.vector.scalar_tensor_tensor(
            out=term[0:NT, :],
            in0=trT[0:NT, STOP : STOP + 1],
            scalar=mx[0:NT, 0:1],
            in1=feats[0:NT, steps - 1 : steps],
            op0=OP.add,
            op1=OP.add,
        )
        # best tag one-hot
        t32 = st.tile([32, 32], F32)
        nc.gpsimd.memset(t32[:], NEG)
        nc.vector.tensor_copy(t32[:, 0:1], term[:])
        tT = st.tile([32, 32], F32)
        nc.vector.transpose(tT[:], t32[:])
        mxt = st.tile([32, 8], F32)
        nc.vector.max(mxt[0:1, :], tT[0:1, 0:NT])
        onesf = st.tile([1, NT], F32)
        nc.gpsimd.memset(onesf[:], 1.0)
        pmx = psum.tile([32, 1], F32, space="PSUM", tag="pmx")
        nc.tensor.matmul(pmx[0:NT, :], onesf[0:1, 0:NT], mxt[0:1, 0:1], start=True, stop=True)
        mxb = st.tile([32, 1], F32)
        nc.vector.tensor_copy(mxb[0:NT, :], pmx[0:NT, :])
        pathOH = st.tile([32, steps], F32)
        nc.gpsimd.memset(pathOH[:], 0.0)
        nc.vector.tensor_scalar(
            pathOH[0:NT, steps - 1 : steps], term[0:NT, :], mxb[0:NT, 0:1], None,
            OP.is_equal,
        )

        # one-hot backpointer matrices M_all[p, t*20+n] = (bptr[p,t] == n),
        # built in half-chunks so the low half's argmax/one-hot work hides
        # under the high half's backtrace chain.
        iotar = st.tile([32, NT], I32)
        nc.gpsimd.iota(iotar[:], pattern=[[1, NT]], base=0, channel_multiplier=0)
        iotarf = st.tile([32, NT], F32)
        nc.vector.tensor_copy(iotarf[:], iotar[:])
        bpf = st.tile([32, steps], F32)
        mall = st.tile([32, steps * NT], F32)

        def mall_chunk(lo, hi):
            n = hi - lo
            nc.vector.tensor_copy(
                bpf[0:NT, lo:hi],
                bpt[0:NT, 8 * lo : 8 * hi].rearrange("p (t e) -> p t e", e=8)[:, :, 0],
            )
            nc.vector.tensor_tensor(
                out=mall[0:NT, lo * NT : hi * NT].rearrange("p (t n) -> p t n", n=NT),
                in0=bpf[0:NT, lo:hi].rearrange("p (t o) -> p t o", o=1)
                    .broadcast_to([NT, n, NT]),
                in1=iotarf[0:NT, :].rearrange("p (o n) -> p o n", o=1)
                    .broadcast_to([NT, n, NT]),
                op=OP.is_equal,
            )

        def bt_chain(lo, hi, filler=None):
            if "backtrace" in _skip:
                return
            for t in range(hi - 2, lo - 2, -1):
                if t < 0:
                    break
                pv = psum.tile([32, 1], F32, space="PSUM", tag="pv")
                nc.tensor.matmul(
                    pv[0:NT, :],
                    mall[0:NT, (t + 1) * NT : (t + 2) * NT],
                    pathOH[0:NT, t + 1 : t + 2],
                    start=True, stop=True,
                )
                # ScalarE copy keeps the DVE free for the interleaved argmaxes
                nc.scalar.copy(pathOH[0:NT, t : t + 1], pv[0:NT, :])
                if filler is not None:
                    next(filler, None)

        def maxidx_gen(lo, hi):
            # one deferred argmax per yield, interleaved between chain links
            for t in range(lo, min(hi, crf_steps)):
                nc.vector.max_index(
                    bpt[0:NT, 8 * t : 8 * t + 8],
                    mxhist[0:NT, 8 * t : 8 * t + 8],
                    schist[0:NT, 32 * t : 32 * t + NT],
                )
                yield t

        half = steps // 2
        maxidx_batch(half, steps)
        mall_chunk(half, steps)
        bt_chain(half, steps, filler=maxidx_gen(0, half))
        mall_chunk(0, half)
        bt_chain(0, half)

        # path_int[t] = iota . pathOH[:, t]
        iotac = st.tile([32, 1], I32)
        nc.gpsimd.iota(iotac[:], pattern=[[0, 1]], base=0, channel_multiplier=1)
        iotacf = st.tile([32, 1], F32)
        nc.vector.tensor_copy(iotacf[:], iotac[:])
        pp = psum.tile([32, steps], F32, space="PSUM", tag="pp")
        nc.tensor.matmul(pp[0:1, :], iotacf[0:NT, :], pathOH[0:NT, :], start=True, stop=True)
        path_sb = st.tile([1, steps], I32)
        nc.vector.tensor_copy(path_sb[:], pp[0:1, :])
        nc.sync.dma_start(path_d[:, :], path_sb[:])
    nc.compile()
    return nc


# --------------------------------------------------------------------------
# host glue
# --------------------------------------------------------------------------
def _prep_l1_inputs(sentence, embed_table, wih, bih, bhh):
    sent = np.ascontiguousarray(
        np.asarray(sentence, np.int32).reshape(4, 128).T
    )
    ins = {"emb": np.asarray(embed_table, np.float32), "sent": sent}
    for d in ("f", "b"):
        w = np.asarray(wih[d], np.float32)[_PERM]          # [2048, 300]
        b = (np.asarray(bih[d], np.float32) + np.asarray(bhh[d], np.float32))[_PERM]
        wT = np.ascontiguousarray(w.T)                     # [300, 2048]
        ins[f"wA_{d}"] = np.ascontiguousarray(
            np.concatenate([wT[0:128], wT[128:256]], axis=1)
        )
        ins[f"wB_{d}"] = np.ascontiguousarray(wT[256:300])
        ins[f"wC_{d}"] = np.ascontiguousarray(b[None, :])
    return ins


def _prep_l2_inputs(xprojT, whh, h0, c0):
    # xprojT: [2048, 512] (gate-permuted rows, bias included)
    import ml_dtypes
    rdt = np.float32 if RECUR_DT == F32 else ml_dtypes.bfloat16
    w = np.asarray(whh, np.float32)[_PERM]                 # [2048, 512]
    wT = np.ascontiguousarray(w.T)                         # [512, 2048]
    wpack = np.ascontiguousarray(
        wT.reshape(NK, 128, G4).transpose(1, 0, 2).reshape(128, NK * G4)
    ).astype(rdt)
    xp = np.ascontiguousarray(
        xprojT.reshape(NM, 128, L).transpose(1, 2, 0).reshape(128, L * NM)
    )
    h0c = np.ascontiguousarray(
        np.asarray(h0, np.float32).reshape(NK, 128).T
    ).astype(rdt)
    c0c = np.ascontiguousarray(np.asarray(c0, np.float32).reshape(NK, 128).T)
    return {"wpack": wpack, "xproj": xp, "h0c": h0c, "c0c": c0c}


def _prep_l3_inputs(hTf, hTb_scan, wout, bout, transitions):
    # hTf / hTb_scan: [128, 4*512]; backward scan is in scan order (reversed time)
    blocks = [hTf[:, j * L : (j + 1) * L] for j in range(NK)]
    blocks += [hTb_scan[:, j * L : (j + 1) * L][:, ::-1] for j in range(NK)]
    hcat = np.ascontiguousarray(np.concatenate(blocks, axis=1))
    woT = np.ascontiguousarray(np.asarray(wout, np.float32).T)  # [1024, 20]
    wop = np.ascontiguousarray(
        np.concatenate([woT[j * 128 : (j + 1) * 128] for j in range(8)], axis=1)
    )
    trTp = np.zeros((32, 32), np.float32)
    trTp[0:NT, 0:NT] = np.asarray(transitions, np.float32).T
    fvi = np.zeros((32, 1), np.float32)
    fvi[0:NT, 0] = NEG
    fvi[START, 0] = 0.0
    import ml_dtypes
    return {
        "hcat": hcat.astype(ml_dtypes.bfloat16),
        "woutp": wop.astype(ml_dtypes.bfloat16),
        "bout": np.ascontiguousarray(
            np.asarray(bout, np.float32)[None, :]).astype(ml_dtypes.bfloat16),
        "transTp": trTp,
        "fvinit": fvi,
    }


def _get(name, builder):
    if name not in _CACHE:
        _CACHE[name] = builder()
    return _CACHE[name]


def _prep_l12_inputs(sentence, embed_table, wih, bih, bhh, whh, h0, c0, reverse):
    import ml_dtypes
    rdt = np.float32 if RECUR_DT == F32 else ml_dtypes.bfloat16
    s = np.asarray(sentence, np.int32)
    if reverse:
        s = s[::-1]
    ins = {
        "emb": np.asarray(embed_table, np.float32),
        "sent": np.ascontiguousarray(s.reshape(4, 128).T),
    }
    w = np.asarray(wih, np.float32)[_PERM]                 # [2048, 300]
    b = (np.asarray(bih, np.float32) + np.asarray(bhh, np.float32))[_PERM]
    wT = np.ascontiguousarray(w.T)                         # [300, 2048]
    ins["wA"] = np.ascontiguousarray(
        np.concatenate([wT[0:128], wT[128:256]], axis=1)).astype(ml_dtypes.bfloat16)
    ins["wB"] = np.ascontiguousarray(wT[256:300]).astype(ml_dtypes.bfloat16)
    ins["wC"] = np.ascontiguousarray(b[None, :]).astype(ml_dtypes.bfloat16)
    wh = np.asarray(whh, np.float32)[_PERM]                # [2048, 512]
    whT = np.ascontiguousarray(wh.T)                       # [512, 2048]
    ins["wpack"] = np.ascontiguousarray(
        whT.reshape(NK, 128, G4).transpose(1, 0, 2).reshape(128, NK * G4)
    ).astype(rdt)
    ins["h0c"] = np.ascontiguousarray(
        np.asarray(h0, np.float32).reshape(NK, 128).T
    ).astype(rdt)
    ins["c0c"] = np.ascontiguousarray(np.asarray(c0, np.float32).reshape(NK, 128).T)
    return ins


def kernel(sentence, embed_table, w_ih_f, w_hh_f, b_ih_f, b_hh_f,
           w_ih_b, w_hh_b, b_ih_b, b_hh_b, h0, c0, w_out, b_out, transitions):
    h0 = np.asarray(h0, np.float32)
    c0 = np.asarray(c0, np.float32)

    # ---- L12: per-core gather + input projection + LSTM recurrence
    nc2 = _get("l12", build_l2)
    in_f = _prep_l12_inputs(sentence, embed_table, w_ih_f, b_ih_f, b_hh_f,
                            w_hh_f, h0[0], c0[0], reverse=False)
    in_b = _prep_l12_inputs(sentence, embed_table, w_ih_b, b_ih_b, b_hh_b,
                            w_hh_b, h0[1], c0[1], reverse=True)
    r2 = run_bass_kernel_spmd(nc2, [in_f, in_b], core_ids=[0, 1]).results
    hTf = r2[0]["hT_out"]       # [128, 2048]
    hTb_scan = r2[1]["hT_out"]  # backward scan order

    # ---- L3: feats + viterbi + backtrace
    nc3 = _get("l3", build_l3)
    ins3 = _prep_l3_inputs(hTf, hTb_scan, w_out, b_out, transitions)
    r3 = run_bass_kernel_spmd(nc3, [ins3], core_ids=[0]).results[0]
    return np.ascontiguousarray(r3["path"].reshape(L)).astype(np.int32)

